# revision 10
# baseline (speedup 1.0000x reference)
"""EvidenceLevelAttention (additive attention GNN message passing) on 8 trn2 cores.

Math per batch b (B=8, N=256, H=300):
    ai = h @ W0a.T ; aj = h @ W0b.T                     (W0a = W0[:, :H], W0b = W0[:, H:])
    p[i, j] = w1 . relu(ai[i] + aj[j] + b0)  (+ b1, dropped: softmax shift-invariant)
    a = softmax(p, axis=-1) ;  y = a @ h

Data-parallel: core c computes batch c. Heavy math in fp16 with fp32 PSUM
accumulation.

Layout: hidden dim k (300 -> padded 384 = 3x128) on partitions for the pairwise
phase, so the per-i bias (aiT[:, i] + b0) is a per-partition scalar: one fused
DVE tensor_scalar(add, max) per (i, k-block) computes relu(ajT + bias) for all
256 j. TensorE then contracts with w1 by loading T as the stationary operand
(128 j columns at a time) and streaming w1 as the 1-wide moving operand, so
p^T[j, i] accumulates as full 128-partition psum columns. Softmax needs no
transposes: p is O(1) here so exp(p) is computed without max-subtraction, row
sums come from a ones-matmul, and 1/s is applied as a per-partition scale on
the final output u = e^T.T @ h.
"""

import numpy as np

import concourse.bass as bass
import concourse.mybir as mybir
import concourse.tile as tile
from concourse import bacc
from concourse.bass_utils import run_bass_kernel_spmd
from concourse.masks import make_identity

B, N, H = 8, 256, 300
HB = 3          # hidden-dim blocks of 128
HP = HB * 128   # padded hidden dim
NB = 2          # row blocks of 128
F32 = mybir.dt.float32
F16 = mybir.dt.float16
ACT_EVERY = 3   # legacy knob (unused when ENGINE_PATTERN set)
ENGINE_PATTERN = ["V", "A", "V", "V", "G", "V"]  # full-block relu engine rotation
TAIL_PATTERN = ["G", "V", "A", "V", "V", "A"]  # tail-op rotation (ttt is its own tile, so a different engine keeps single-producer tiles)
N_I = N         # phase-B iteration count (reduced for calibration benches)
REPEAT_B = 1    # timing-only: repeat phase-B loop (calibration; >1 is wrong math)
B_MODE = "full"  # timing-only: "full" | "ew_only" | "mm_only" (non-full is wrong math)
SKIP_RELU = False   # timing-only: single-op tensor_scalar (wrong math)
SKIP_MM = False     # timing-only: skip phase-B matmuls (wrong math)
FD_TEST = None      # timing-only: shrink elementwise free dim (wrong math)
T_BUFS = 24

_CACHE = {}


def _emit(nc):
    f32, f16 = F32, F16
    Alu = mybir.AluOpType
    Relu = mybir.ActivationFunctionType.Relu
    Exp = mybir.ActivationFunctionType.Exp

    h_in = nc.dram_tensor("h", [N, H], f32, kind="ExternalInput")
    w0_in = nc.dram_tensor("w0", [H, 2 * H], f32, kind="ExternalInput")
    b0_in = nc.dram_tensor("b0", [H], f32, kind="ExternalInput")
    w1_in = nc.dram_tensor("w1", [H], f32, kind="ExternalInput")
    y_out = nc.dram_tensor("y", [N, H], f32, kind="ExternalOutput")

    with tile.TileContext(nc) as tc:
        with (
            tc.tile_pool(name="const", bufs=1) as const,
            tc.tile_pool(name="work", bufs=2) as work,
            tc.tile_pool(name="tpool", bufs=T_BUFS) as tpool,
            tc.tile_pool(name="psA", bufs=2, space="PSUM") as psA,
            tc.tile_pool(name="psT", bufs=2, space="PSUM") as psT,
            tc.tile_pool(name="psP", bufs=1, space="PSUM") as psP,
            tc.tile_pool(name="psO", bufs=2, space="PSUM") as psO,
        ):
            # ---------------- phase 0: loads, casts, transposes ----------------
            # h rows, fp32 then fp16 (k-padded with zeros)
            h_f32 = [const.tile([128, H], f32, name=f"h_f32_{k}") for k in range(NB)]
            h_f16 = [const.tile([128, HP], f16, name=f"h_f16_{k}") for k in range(NB)]
            for ib in range(NB):
                nc.sync.dma_start(out=h_f32[ib], in_=h_in[ib * 128:(ib + 1) * 128, :])
                nc.vector.memset(h_f16[ib][:, H:HP], 0.0)
                nc.vector.memset(h_f16[ib][:, H:H + 1], 1.0)  # ones col for fused row-sum
                nc.vector.tensor_scalar(out=h_f16[ib][:, 0:H], in0=h_f32[ib], scalar1=0.0, scalar2=None, op0=Alu.add)

            # hT[hb]: [128 h, 256 n]  (PE transpose of fp16 tiles)
            ident = const.tile([128, 128], f16)
            make_identity(nc, ident)
            hT = [const.tile([128, N], f16, name=f"hT_{k}") for k in range(HB)]
            ncopy = 0
            for hb in range(HB):
                for ib in range(NB):
                    pst = psT.tile([128, 128], f16, tag="tr")
                    nc.tensor.transpose(
                        pst, h_f16[ib][:, hb * 128:(hb + 1) * 128], ident,
                    )
                    dst_sl = hT[hb][:, ib * 128:(ib + 1) * 128]
                    if ncopy % 2 == 0:
                        nc.vector.tensor_scalar(out=dst_sl, in0=pst, scalar1=0.0, scalar2=None, op0=Alu.add)
                    else:
                        nc.scalar.copy(dst_sl, pst)
                    ncopy += 1

            # W0, k-blocked rows, columns split [W0a | pad | W0b | pad], fp16
            w0_f16 = []
            for kb in range(HB):
                k0 = kb * 128
                ksz = min(H, k0 + 128) - k0
                t32 = work.tile([128, 2 * H], f32, tag="w0scratch")
                tf = const.tile([128, 2 * HP], f16, name=f"w0f16_{kb}")
                nc.sync.dma_start(out=t32[0:ksz, :], in_=w0_in[k0:k0 + ksz, :])
                nc.vector.memset(tf, 0.0)
                nc.vector.tensor_scalar(out=tf[0:ksz, 0:H], in0=t32[0:ksz, 0:H], scalar1=0.0, scalar2=None, op0=Alu.add)
                nc.vector.tensor_scalar(out=tf[0:ksz, HP:HP + H], in0=t32[0:ksz, H:2 * H], scalar1=0.0, scalar2=None, op0=Alu.add)
                w0_f16.append(tf)

            # W0aT/W0bT[hb]: [128 h, 384 k] via PE transpose (128x128 blocks)
            w0aT = [const.tile([128, HP], f16, name=f"w0aT_{k}") for k in range(HB)]
            w0bT = [const.tile([128, HP], f16, name=f"w0bT_{k}") for k in range(HB)]
            for half, dst in ((0, w0aT), (1, w0bT)):
                for hb in range(HB):
                    for kb in range(HB):
                        pst = psT.tile([128, 128], f16, tag="tr")
                        nc.tensor.transpose(
                            pst,
                            w0_f16[kb][:, half * HP + hb * 128: half * HP + (hb + 1) * 128],
                            ident,
                        )
                        dst_sl = dst[hb][:, kb * 128:(kb + 1) * 128]
                        if ncopy % 2 == 0:
                            nc.vector.tensor_scalar(out=dst_sl, in0=pst, scalar1=0.0, scalar2=None, op0=Alu.add)
                        else:
                            nc.scalar.copy(dst_sl, pst)
                        ncopy += 1

            # b0 (fp32) and w1 (fp16) as per-partition columns over k-blocks
            b0c = [const.tile([128, 1], f32, name=f"b0c_{k}") for k in range(HB)]
            w1c = [const.tile([128, 1], f16, name=f"w1c_{k}") for k in range(HB)]
            for kb in range(HB):
                k0 = kb * 128
                ksz = min(H, k0 + 128) - k0
                w1f = work.tile([128, 1], f32, tag="w1scratch")
                nc.vector.memset(b0c[kb], 0.0)
                nc.vector.memset(w1c[kb], 0.0)
                nc.sync.dma_start(out=b0c[kb][0:ksz, 0:1], in_=b0_in[k0:k0 + ksz])
                nc.sync.dma_start(out=w1f[0:ksz, 0:1], in_=w1_in[k0:k0 + ksz])
                nc.vector.tensor_scalar(out=w1c[kb][0:ksz, :], in0=w1f[0:ksz, :], scalar1=0.0, scalar2=None, op0=Alu.add)

            # ---------------- phase A: aib = aiT + b0 (fp32), ajT (fp16) -------
            aib = [const.tile([128, N], f32, name=f"aib_{k}") for k in range(HB)]
            ajT = [const.tile([128, N], f16, name=f"ajT_{k}") for k in range(HB)]
            for wT, dst, is_ai in ((w0aT, aib, True), (w0bT, ajT, False)):
                for kb in range(HB):
                    ps = psA.tile([128, N], f32, tag="A")
                    for hb in range(HB):
                        nc.tensor.matmul(
                            ps,
                            lhsT=wT[hb][:, kb * 128:(kb + 1) * 128],
                            rhs=hT[hb],
                            start=(hb == 0),
                            stop=(hb == HB - 1),
                        )
                    if is_ai:
                        nc.vector.tensor_scalar(
                            out=dst[kb], in0=ps, scalar1=b0c[kb], scalar2=None,
                            op0=Alu.add,
                        )
                    else:
                        nc.vector.tensor_scalar(out=dst[kb], in0=ps, scalar1=0.0, scalar2=None, op0=Alu.add)

            # Tail-pair setup: k-block 2 has only 44 real rows, so two queries'
            # tails share one 108-partition op (rows 0:44 = query i, 64:108 =
            # query i+1 via a column-shifted bias layout).
            KT = H - 2 * 128  # 44
            ajT_tail2 = const.tile([128, N], f16)
            aib_tail2 = const.tile([128, N], f32)
            w1tail2col = const.tile([128, 2], f16)
            nc.vector.memset(ajT_tail2, 0.0)
            nc.vector.memset(aib_tail2, 0.0)
            nc.vector.memset(w1tail2col, 0.0)
            nc.vector.tensor_scalar(out=ajT_tail2[0:KT, :], in0=ajT[2][0:KT, :],
                                    scalar1=0.0, scalar2=None, op0=Alu.add)
            nc.vector.tensor_scalar(out=ajT_tail2[64:64 + KT, :], in0=ajT[2][0:KT, :],
                                    scalar1=0.0, scalar2=None, op0=Alu.add)
            nc.vector.tensor_scalar(out=aib_tail2[0:KT, :], in0=aib[2][0:KT, :],
                                    scalar1=0.0, scalar2=None, op0=Alu.add)
            nc.vector.tensor_scalar(out=aib_tail2[64:64 + KT, 0:N - 1],
                                    in0=aib[2][0:KT, 1:N],
                                    scalar1=0.0, scalar2=None, op0=Alu.add)
            nc.vector.tensor_scalar(out=w1tail2col[0:KT, 0:1], in0=w1c[2][0:KT, :],
                                    scalar1=0.0, scalar2=None, op0=Alu.add)
            nc.vector.tensor_scalar(out=w1tail2col[64:64 + KT, 1:2], in0=w1c[2][0:KT, :],
                                    scalar1=0.0, scalar2=None, op0=Alu.add)

            # ------- phase B: pT[j, i] columns = w1 . relu(ajT + aib[:, i]) ----
            pT = [psP.tile([128, N], f32, name=f"pT_{jb}") for jb in range(NB)]
            if SKIP_MM:
                nc.vector.memset(pT[1], 0.0)
            opc = 0
            if B_MODE == "ew_only":
                for jb in range(NB):
                    nc.vector.memset(pT[jb], 0.0)
            mm_tt = mm_ttt = None
            if B_MODE == "mm_only":
                mm_tt = tpool.tile([128, 4 * N], f16, tag="T")
                mm_ttt = tpool.tile([128, N], f16, tag="Tt")
                nc.vector.memset(mm_tt, 0.0)
                nc.vector.memset(mm_ttt, 0.0)
            for i0 in [i for _ in range(REPEAT_B) for i in range(0, N_I, 2)]:
                # 4 full-block ops (2 queries x k-blocks 0,1) + 1 shared tail op
                if B_MODE == "mm_only":
                    tt, ttt = mm_tt, mm_ttt
                else:
                    tt = tpool.tile([128, 4 * N], f16, tag="T")
                    ttt = tpool.tile([128, N], f16, tag="Tt")
                ops = [(q, kb) for q in range(2) for kb in range(2)] + [(2, 2)]
                if B_MODE == "mm_only":
                    ops = []
                pair_sel = ENGINE_PATTERN[(i0 // 2) % len(ENGINE_PATTERN)]
                tail_sel = (TAIL_PATTERN[(i0 // 2) % len(TAIL_PATTERN)]
                            if TAIL_PATTERN else pair_sel)
                for q, kb in ops:
                    if q == 2:
                        out_sl, in_sl = ttt[:, :], ajT_tail2
                        bias = aib_tail2[:, i0:i0 + 1]
                    else:
                        out_sl = tt[:, (q * 2 + kb) * N:(q * 2 + kb + 1) * N]
                        in_sl = ajT[kb]
                        bias = aib[kb][:, i0 + q:i0 + q + 1]
                    sel = tail_sel if q == 2 else pair_sel
                    opc += 1
                    if sel == "A":
                        nc.scalar.activation(out=out_sl, in_=in_sl, func=Relu,
                                             bias=bias, scale=1.0)
                    elif sel == "G":
                        nc.gpsimd.tensor_scalar(out=out_sl, in0=in_sl, scalar1=bias,
                                                scalar2=0.0, op0=Alu.add, op1=Alu.max)
                    else:
                        nc.vector.tensor_scalar(out=out_sl, in0=in_sl, scalar1=bias,
                                                scalar2=0.0, op0=Alu.add, op1=Alu.max)
                jbs = [] if B_MODE == "ew_only" else list(range(1 if SKIP_MM else NB))
                for jb in jbs:
                    # both queries' tails in one FD=2 matmul: the packed ttt
                    # stationary holds q0 rows 0:44, q1 rows 64:108; rhs
                    # column c selects query c's rows via zero-padded w1.
                    # It OPENS the accumulation group for both columns so the
                    # bank sees exactly one start per pair.
                    nc.tensor.matmul(
                        pT[jb][:, i0:i0 + 2],
                        lhsT=ttt[:, jb * 128:jb * 128 + 128],
                        rhs=w1tail2col,
                        start=True,
                        stop=False,
                    )
                    for q in range(2):
                        i = i0 + q
                        for kb in range(2):
                            nc.tensor.matmul(
                                pT[jb][:, i:i + 1],
                                lhsT=tt[:, (q * 2 + kb) * N + jb * 128:
                                        (q * 2 + kb) * N + jb * 128 + 128],
                                rhs=w1c[kb],
                                start=False,
                                stop=(q == 1 and kb == 1),
                            )

            # ---------------- softmax (transposed, no max-subtraction) ---------
            # p is O(1) for this problem (|p| < ~2), so exp never overflows fp16.
            e16 = [const.tile([128, N], f16, name=f"e16_{jb}") for jb in range(NB)]
            for jb in range(NB):
                nc.scalar.activation(out=e16[jb], in_=pT[jb], func=Exp)

            # final: one matmul group per ib gives u = e^T.T @ h AND the row
            # sum s in the appended ones column; y = u * (1/s) per partition
            for ib in range(NB):
                pso = psO.tile([128, H + 1], f32, tag="O")
                for jb in range(NB):
                    nc.tensor.matmul(
                        pso,
                        lhsT=e16[jb][:, ib * 128:(ib + 1) * 128],
                        rhs=h_f16[jb][:, 0:H + 1],
                        start=(jb == 0),
                        stop=(jb == NB - 1),
                    )
                rcol = work.tile([128, 1], f32, tag=f"rcol{ib}")
                nc.vector.reciprocal(rcol, pso[:, H:H + 1])
                yt = work.tile([128, H], f32, tag="y")
                nc.vector.tensor_scalar(
                    out=yt, in0=pso[:, 0:H], scalar1=rcol, scalar2=None, op0=Alu.mult,
                )
                nc.sync.dma_start(out=y_out[ib * 128:(ib + 1) * 128, :], in_=yt)
    return nc


def build_nc():
    nc = bacc.Bacc("TRN2", target_bir_lowering=False, debug=False, num_devices=B)
    _emit(nc)
    nc.compile()
    return nc


def _get_nc():
    if "nc" not in _CACHE:
        _CACHE["nc"] = build_nc()
    return _CACHE["nc"]


def kernel(h_prev, W0, b0, W1, b1, **_ignored):
    del b1  # softmax is invariant to the scalar output bias
    h_prev = np.asarray(h_prev, np.float32)
    W0 = np.asarray(W0, np.float32)
    b0 = np.asarray(b0, np.float32).reshape(H)
    w1 = np.asarray(W1, np.float32).reshape(H)
    assert h_prev.shape == (B, N, H), h_prev.shape

    nc = _get_nc()
    in_maps = [
        {"h": np.ascontiguousarray(h_prev[c]), "w0": W0, "b0": b0, "w1": w1}
        for c in range(B)
    ]
    res = run_bass_kernel_spmd(nc, in_maps, core_ids=list(range(B)))
    return np.stack([res.results[c]["y"] for c in range(B)], axis=0).astype(np.float32)

